# revision 3
# baseline (speedup 1.0000x reference)
"""Trainium2 Bass kernel for nn_ContrastiveLoss (sparse cross-attention t2i loss).

The loss needs, per (caption t, image c) pair, a thresholded softmax
attention over words x regions, then cosine scores and a diagonal margin
reduction. Everything downstream of the raw word-region score matrix is
tiny (sub-GFLOP, done on host in f32); the device computes the only heavy
part and ships it in fp16:

    raw[w, bp] = s[w] . im[bp]        (words x (16 img x 36 regions), D=1024)
    even chunks: raw                  (DVE copy; host applies the leak)
    odd  chunks: LeakyReLU_0.1(raw)   (Activation engine, Prelu)

Sharding: 2D grid over 8 cores = 4 image-blocks (16 images = 576 cols) x
2 caption-halves (32 captions, greedily balanced by total length). Words of
each half are packed contiguously (captions may span chunk boundaries - the
host tail regroups them) into 8 chunks of 128 partition rows. The score
matmul runs in fp8(e4m3) with DoubleRow perf mode (2 contraction subtiles
per pass) after a PE p-state warmup that overlaps the input DMA latency.

Each chunk is converted PSUM->fp16 by a single engine (DVE for even chunks,
ACT for odd) in one instruction, into its own tile, shipped by its own DMA:
the tile framework chains readers/writers of shared tiles and entangles
semaphore thresholds across engines sharing a chunk, so full decoupling
is what pipelines cleanly. The caption stream is issued on two independent
DGE paths (Pool SWDGE + SP HWDGE) as single-chunk transfers, labeled in
expected arrival order so the conversion chain follows the stream.

Host tail (numpy, f32): leak fix-up for the raw (even-chunk) rows, group
norms over regions, xhat = leak/(||leak||+eps), E = exp(9 xhat),
per-caption softmax denominators, threshold sparsify, w12 from the raw
scores, w2 via the K = s s^T trick, cosine, sorted-region mean, diagonal
margin loss. Validated at rel_err ~3e-4 vs the fp32 reference (the fp8
matmul dominates the error; tolerance is 2e-2).
"""

import numpy as np
import ml_dtypes
from contextlib import ExitStack

import concourse.bass as bass
import concourse.bacc as bacc
import concourse.tile as tile
from concourse import mybir, bass_utils

F32 = mybir.dt.float32
F16 = mybir.dt.float16
BF16 = mybir.dt.bfloat16
FP8 = mybir.dt.float8e4
AF = mybir.ActivationFunctionType
OP = mybir.AluOpType

B = 64          # batch (captions == images)
P = 36          # regions per image
D = 1024        # feature dim
BP = B * P      # 2304 score columns
NCORES = 8
NIMG = 4        # image-block split
NCAP = 2        # caption-half split
COLS = BP // NIMG        # 576 columns per core
KP = 4                   # DoubleRow k-pairs (each covers 256 of D)
XA = 272                 # ACT converts cols [0, XA), DVE cols [XA, 576)
XB = COLS - XA
EPS = 1e-8
MARGIN = 2e-1
LAM = 9.0
NWARM = 8                # PE p-state warmup matmuls
B_FIRST = True           # emit psB matmul group before psA
PSPA_BUFS = 3
PSPB_BUFS = 3
WARM_OWN = True          # warmups in a dedicated psum bank
IMB_FIRST = False        # load imB before imA
INS_MODE = "two_path"    # input issue scheme


def _fp8(x):
    return np.asarray(x, np.float32).astype(ml_dtypes.float8_e4m3)


def _build_device_program(nt):
    """nt = number of chunk pairs (chunks = 2*nt, 128 packed word rows each)."""
    nc = bacc.Bacc("TRN2", target_bir_lowering=False, debug=False)

    imTA = nc.dram_tensor("imTA", [128, KP, 2, XA], FP8, kind="ExternalInput")
    imTB = nc.dram_tensor("imTB", [128, KP, 2, XB], FP8, kind="ExternalInput")
    sT = nc.dram_tensor("sT", [2 * nt, 128, KP, 2, 128], FP8, kind="ExternalInput")
    outs = {}
    for m in range(nt):
        outs[f"lkA{m}"] = nc.dram_tensor(
            f"lkA{m}", [128, 2, XA], F16, kind="ExternalOutput")
        outs[f"lkB{m}"] = nc.dram_tensor(
            f"lkB{m}", [128, 2, XB], F16, kind="ExternalOutput")
    with tile.TileContext(nc) as tc:
        with ExitStack() as ctx:
            _body(ctx, tc, nt, imTA, imTB, sT, outs)
    nc.compile()
    return nc


def _body(ctx, tc, nt, imTA, imTB, sT, outs):
    nc = tc.nc
    nch = 2 * nt

    consts = ctx.enter_context(tc.tile_pool(name="consts", bufs=1))
    sp = ctx.enter_context(tc.tile_pool(name="sp", bufs=1))
    ap_ = ctx.enter_context(tc.tile_pool(name="ap", bufs=nt))
    bp_ = ctx.enter_context(tc.tile_pool(name="bp", bufs=nt))
    pspA = ctx.enter_context(tc.tile_pool(name="pspA", bufs=PSPA_BUFS, space="PSUM"))
    pspB = ctx.enter_context(tc.tile_pool(name="pspB", bufs=PSPB_BUFS, space="PSUM"))

    # PE p-state warmup (memset on the otherwise idle DVE queue; overlaps
    # the input DMA latency)
    warm = consts.tile([128, 288], BF16)
    nc.vector.memset(warm.bitcast(mybir.dt.int16), 0)
    if WARM_OWN:
        wpsp = ctx.enter_context(tc.tile_pool(name="wpsp", bufs=1, space="PSUM"))
        wps = wpsp.tile([128, 512], F32, name="warm_ps")
    else:
        wps = pspA.tile([128, 512], F32, name="psA", tag="psA")
    for _ in range(NWARM):
        nc.tensor.matmul(wps[:, :288], lhsT=warm[:, :128], rhs=warm)

    # ---- input DMAs on two independent issue paths, single-chunk
    # transfers, labeled in expected arrival order ----
    imA_sb = consts.tile([128, KP, 2, XA], FP8, name="imA")
    imB_sb = consts.tile([128, KP, 2, XB], FP8, name="imB")
    sT_sb = [None] * nch

    def load_sT(c, q):
        t = sp.tile([128, KP, 2, 128], FP8, name=f"sT{c}", tag=f"sT{c}")
        q.dma_start(t, sT.ap()[c])
        sT_sb[c] = t

    # two independent issue paths (Pool SWDGE + SP HWDGE), single-chunk
    # transfers, labeled in expected arrival order
    if IMB_FIRST:
        nc.sync.dma_start(imB_sb, imTB.ap())
        nc.sync.dma_start(imA_sb, imTA.ap())
    else:
        nc.sync.dma_start(imA_sb, imTA.ap())
        nc.sync.dma_start(imB_sb, imTB.ap())
    if nch == 8:
        # arrival-order-tuned interleave of the two issue paths
        pool_chunks, sp_chunks = (0, 2, 4, 7), (1, 3, 5, 6)
    else:
        pool_chunks = tuple(range(0, nch, 2))
        sp_chunks = tuple(range(1, nch, 2))
    for c in pool_chunks:
        load_sT(c, nc.gpsimd)
    for c in sp_chunks:
        load_sT(c, nc.sync)

    at = bt = None
    for c in range(nch):
        m, c2 = divmod(c, 2)
        psA = (wps if (not WARM_OWN and c == 0)
               else pspA.tile([128, 512], F32, name="psA", tag="psA"))
        psB = pspB.tile([128, 512], F32, name="psB", tag="psB")
        # sequential accumulation groups: interleaving lets the scheduler
        # entangle the conversion's sem threshold with later chunks' matmuls
        groups = [(psA, imA_sb), (psB, imB_sb)]
        if B_FIRST:
            groups.reverse()
        for ps, imx in groups:
            for kp in range(KP):
                nc.tensor.matmul(
                    ps[:, :imx.shape[-1]], lhsT=sT_sb[c][:, kp], rhs=imx[:, kp],
                    start=(kp == 0), stop=(kp == KP - 1),
                    perf_mode=mybir.MatmulPerfMode.DoubleRow,
                )
        if c2 == 0:
            at = ap_.tile([128, 2, XA], F16, name=f"A{m}", tag="A")
            bt = bp_.tile([128, 2, XB], F16, name=f"B{m}", tag="B")
        # per-chunk conversion; separate psum + output tiles per engine (the
        # tile framework chains readers/writers of any shared tile)
        nc.scalar.activation(at[:, c2], psA[:, :XA], AF.Prelu, alpha=0.1)
        nc.vector.tensor_scalar(bt[:, c2], psB[:, :XB], 0.0, None, op0=OP.add)
        if c2 == 1:
            # ship the finished pair; B0-B2 on the Pool SWDGE queue, A0-A3
            # and the final B on the SP HWDGE queue
            nc.sync.dma_start(outs[f"lkA{m}"].ap(), at)
            qb = nc.sync if m == nt - 1 else nc.gpsimd
            qb.dma_start(outs[f"lkB{m}"].ap(), bt)


_CACHE = {}


def _get_program(nt):
    if nt not in _CACHE:
        _CACHE[nt] = _build_device_program(nt)
    return _CACHE[nt]


def _balance_halves(cl):
    """Greedy partition of the 64 captions into 2 halves of 32, balancing
    total word count. Returns halves[j] = list of caption indices."""
    order = np.argsort(-cl, kind="stable")
    halves = [[], []]
    tot = [0, 0]
    for t in order:
        j = 0 if (tot[0] <= tot[1] and len(halves[0]) < 32) or len(halves[1]) >= 32 else 1
        halves[j].append(int(t))
        tot[j] += int(cl[t])
    return halves


def _host_inputs(im, s, cl):
    # image blocks -> imT per image-core-row, fp8, DoubleRow layout
    imTs = []
    for i in range(NIMG):
        imf = im[16 * i:16 * (i + 1)].reshape(COLS, D).T  # (D, 576)
        arr = imf.reshape(KP, 2, 128, COLS)               # d = kp*256+e*128+p
        arr = arr.transpose(2, 0, 1, 3)                   # [128, KP, 2, COLS]
        imTs.append((_fp8(np.ascontiguousarray(arr[..., :XA])),
                     _fp8(np.ascontiguousarray(arr[..., XA:]))))

    halves = _balance_halves(cl)
    tots = [int(sum(int(cl[t]) for t in halves[j])) for j in range(NCAP)]
    nch = (max(tots) + 127) // 128
    nch += nch % 2
    nt = nch // 2

    sTs = []
    for j in range(NCAP):
        rows = np.zeros((nch * 128, D), np.float32)
        o = 0
        for t in halves[j]:
            l = int(cl[t])
            rows[o:o + l] = s[t, :l]
            o += l
        arr = rows.reshape(nch, 128, KP, 2, 128)   # [c, row, kp, e, p]
        arr = arr.transpose(0, 4, 2, 3, 1)         # [c, 128p, kp, e, row]
        sTs.append(_fp8(np.ascontiguousarray(arr)))

    in_maps = []
    for c in range(NCORES):
        i, j = divmod(c, NCAP)
        in_maps.append({"imTA": imTs[i][0], "imTB": imTs[i][1], "sT": sTs[j]})
    return nt, in_maps, halves


_RAWCOL = None


def _raw_cols():
    global _RAWCOL
    if _RAWCOL is None:
        _RAWCOL = (np.arange(BP) % COLS) >= XA
    return _RAWCOL


def _host_tail(im, s, cl, leaks, halves):
    """leaks[j] = (nrows, 2304) f32; leak already applied everywhere."""
    imf = im.reshape(BP, D)
    w1 = np.sqrt(np.sum(imf * imf, axis=1, dtype=np.float32))
    scores = np.empty((B, B), np.float32)
    for j in range(NCAP):
        lkj = leaks[j]
        o = 0
        for t in halves[j]:
            l = int(cl[t])
            leak = lkj[o:o + l]
            o += l
            nsum = (leak * leak).reshape(l, B, P).sum(axis=2)  # (l, 64)
            norm = np.sqrt(nsum)
            xhat = leak / (norm + np.float32(EPS)).repeat(P, axis=1)
            E = np.exp(np.float32(LAM) * xhat)
            denom = E.sum(axis=0)                          # (2304,)
            thr = denom / np.float32(l)
            G = np.where(E > thr[None, :], E, 0.0)
            raw = np.where(leak >= 0, leak, np.float32(10.0) * leak)
            w12 = np.einsum("lc,lc->c", G, raw, optimize=True)
            st = s[t, :l]                                  # (l, D)
            K = st @ st.T
            w2 = np.einsum("lc,lc->c", G, K @ G, optimize=True)
            cos = w12 / np.maximum(w1 * np.sqrt(np.maximum(w2, 0.0)), np.float32(EPS))
            cosr = np.sort(cos.reshape(B, P), axis=-1)[:, P // 3:]
            scores[t] = cosr.mean(axis=-1, dtype=np.float32)

    d = np.diag(scores).copy()
    cs = np.maximum(np.float32(MARGIN) + scores - d[:, None], 0.0)
    ci = np.maximum(np.float32(MARGIN) + scores - d[None, :], 0.0)
    np.fill_diagonal(cs, 0.0)
    np.fill_diagonal(ci, 0.0)
    loss = cs.max(axis=1).sum(dtype=np.float32) + ci.max(axis=0).sum(dtype=np.float32)
    return np.asarray(loss, dtype=np.float32)


def kernel(im, s, cap_lens, _profile=False):
    im = np.ascontiguousarray(np.asarray(im, dtype=np.float32))
    s = np.ascontiguousarray(np.asarray(s, dtype=np.float32))
    cl = np.asarray(cap_lens).astype(np.int64)

    nt, in_maps, halves = _host_inputs(im, s, cl)
    nc = _get_program(nt)
    kw = dict(trace=True) if _profile else {}
    res = bass_utils.run_bass_kernel_spmd(
        nc, in_maps, core_ids=list(range(NCORES)), **kw
    )
    # assemble per-half leak matrices [nch*128, 2304]; apply the leak to the
    # raw B columns
    nch = 2 * nt
    leaks = []
    for j in range(NCAP):
        cols = []
        for i in range(NIMG):
            r = res.results[i * NCAP + j]
            blk = np.empty((nch, 128, COLS), np.float32)
            a = np.concatenate(
                [np.asarray(r[f"lkA{m}"]) for m in range(nt)], axis=1)
            b = np.asarray(np.concatenate(
                [np.asarray(r[f"lkB{m}"]) for m in range(nt)], axis=1),
                dtype=np.float32)
            b = np.where(b < 0, np.float32(0.1) * b, b)
            blk[:, :, :XA] = a.transpose(1, 0, 2)
            blk[:, :, XA:] = b.transpose(1, 0, 2)
            cols.append(blk.reshape(-1, COLS))
        leaks.append(np.concatenate(cols, axis=1))
    out = _host_tail(im, s, cl, leaks, halves)
    if _profile:
        return out, res
    return out
